# revision 28
# baseline (speedup 1.0000x reference)
"""Low-rank attention kernel for Trainium2, distributed over 8 NeuronCores.

Math (per batch b):
    u  = q @ Wu            [N, R]
    vp = k @ Wv            [N, R]
    S  = u @ vp.T / sqrt(R)
    out = softmax(S) @ v   [N, D]

Shapes: B=4, N=4096, D=1024, R=32.

Sharding: data-parallel over batch x row-halves -> 8 shards. Core c handles
batch b = c // 2, rows [h*2048, (h+1)*2048) with h = c % 2. Each core gets its
q-shard and the full k/v for its batch. q/k are fed pre-transposed ([D, n]
layout, fp16) so every matmul contraction lands on the partition axis with no
on-device transposes.

Per-core device kernel (all matmuls fp16: 1 col/cycle full PE rate):
  1. uT[R, 2048]  = sum_d Wu[d, :].T qT[d, :]   (K=128 d-tiles, PSUM accum)
     vpT[R, 4096] = sum_d Wv[d, :].T kT[d, :]
  2. flash-style main loop over n-chunks of 256 rows:
       for each m-tile (128 cols): scoresT[m128, n256] = vpT_tile.T @ uT_chunk
       expT = Exp(scoresT / sqrt(R))                       (ScalarE, PSUM->SBUF)
       out_acc[n128, d512] += expT_tile.T @ v_tile         (PSUM accum over m)
       sum_acc[n128, 1]    += expT_tile.T @ ones
     out = out_acc * (1 / sum_acc)   (softmax normalization folded at the end)

To keep the PE warm and hide the kT DMA, the vpT projection is computed
lazily: quarter q of vpT is emitted between 8-m-tile segments of chunk 0,
so chunk-0 AV work overlaps the kT quarter loads. uT chunks 1-3 are emitted
between later main-loop chunks.
"""

import numpy as np

B, N, D, R = 4, 4096, 1024, 32
NLOC = N // 2            # rows per core
RSCALE = float(1.0 / np.sqrt(np.float32(R)))

N_CHUNK = 256            # rows of scores computed per PSUM round
D_HALF = 512             # PSUM bank width in fp32

LAST_RESULT = None       # test.py reads exec_time_ns etc. from here


def _build():
    from concourse import bacc, mybir
    from concourse.tile import TileContext

    f32 = mybir.dt.float32
    f16 = mybir.dt.float16
    EXP = mybir.ActivationFunctionType.Exp

    nc = bacc.Bacc("TRN2", target_bir_lowering=False)

    qT = nc.dram_tensor("qT", [D, NLOC], f16, kind="ExternalInput")
    kT = nc.dram_tensor("kT", [D, N], f16, kind="ExternalInput")
    v = nc.dram_tensor("v", [N, D], f16, kind="ExternalInput")
    wu = nc.dram_tensor("wu", [D, R], f16, kind="ExternalInput")
    wv = nc.dram_tensor("wv", [D, R], f16, kind="ExternalInput")
    o = nc.dram_tensor("o", [NLOC, D], f32, kind="ExternalOutput")

    DT = D // 128         # 8 d-tiles
    UC = NLOC // 512      # 4 uT column-chunks
    MQ = N // 1024        # 4 kT quarters
    NCH = NLOC // N_CHUNK  # 8 main-loop chunks
    MT = N // 128         # 32 m tiles
    VG = 8                # v row-groups of 512
    VPG = N // VG // 128  # 4 m-tiles per v group

    with TileContext(nc) as tc:
        with tc.tile_pool(name="singles", bufs=1) as singles, \
             tc.tile_pool(name="ktp", bufs=3) as ktp, \
             tc.tile_pool(name="qtp", bufs=4) as qtp, \
             tc.tile_pool(name="vpool", bufs=VG - 1) as vpool, \
             tc.tile_pool(name="expp", bufs=6) as expp, \
             tc.tile_pool(name="outp", bufs=3) as outp, \
             tc.tile_pool(name="rpool", bufs=4) as rpool, \
             tc.tile_pool(name="pacc", bufs=5, space="PSUM") as pacc, \
             tc.tile_pool(name="pscore", bufs=2, space="PSUM") as pscore, \
             tc.tile_pool(name="psums", bufs=1, space="PSUM") as psums:

            # ---- constants / projection weights ----
            wu_sb = singles.tile([128, DT, R], f16, tag="wu")
            nc.sync.dma_start(out=wu_sb, in_=wu.rearrange("(t p) r -> p t r", p=128))
            wv_sb = singles.tile([128, DT, R], f16, tag="wv")
            nc.sync.dma_start(out=wv_sb, in_=wv.rearrange("(t p) r -> p t r", p=128))
            ones = singles.tile([128, 2], f16, tag="ones")
            nc.vector.memset(ones, 1.0)

            # uT is replicated on partition groups 0-31 / 32-63 and vpT is
            # distributed even/odd-m-tile across them, so the scores matmuls
            # for an m-tile PAIR run concurrently as two row-tiled (K=32)
            # matmuls on different 32-row strips of the PE array.
            uT = singles.tile([2 * R, NLOC], f16, tag="uT")
            vpT = singles.tile([2 * R, MT // 2, 128], f16, tag="vpT")

            # ---- DMA emission, in order of need (one trigger per load:
            # DMA trigger instructions serialize at ~650ns on the Sync queue)
            def load_qt_chunk(c):
                tile = qtp.tile([128, DT, 512], f16, tag="qt", name=f"qt{c}")
                nc.sync.dma_start(
                    out=tile,
                    in_=qT[:, c * 512:(c + 1) * 512].rearrange(
                        "(t p) c -> p t c", p=128))
                return tile

            def load_kt_quarter(q):
                # kT arrives host-permuted: within each 1024-col quarter, the
                # even m-tiles' 512 columns come first, then the odd ones, so
                # the vp projection's even/odd moving operands are plain
                # slices
                tile = ktp.tile([128, DT, 1024], f16, tag="kt", name=f"kt{q}")
                nc.sync.dma_start(
                    out=tile,
                    in_=kT[:, q * 1024:(q + 1) * 1024].rearrange(
                        "(t p) c -> p t c", p=128))
                return tile

            v_sb = [None] * VG

            def load_v(g):
                vt = vpool.tile([128, VPG, D], f16, tag="v", name=f"v{g}")
                nc.sync.dma_start(
                    out=vt, in_=v[g * 512:(g + 1) * 512, :].rearrange(
                        "(t p) d -> p t d", p=128))
                v_sb[g] = vt

            # group 0 is split in half so the first AV matmul only waits on
            # 0.5MB of v
            def load_v0():
                va = vpool.tile([128, 2, D], f16, tag="v0a", name="v0a", bufs=1)
                nc.sync.dma_start(
                    out=va, in_=v[0:256, :].rearrange("(t p) d -> p t d", p=128))
                vb = vpool.tile([128, 2, D], f16, tag="v0b", name="v0b", bufs=1)
                nc.sync.dma_start(
                    out=vb, in_=v[256:512, :].rearrange("(t p) d -> p t d", p=128))
                v_sb[0] = (va, vb)

            kt_q = [load_kt_quarter(0)]
            qt0 = load_qt_chunk(0)
            load_v0()
            load_v(1)
            kt_q.append(load_kt_quarter(1))
            load_v(2)
            load_v(3)
            kt_q.append(load_kt_quarter(2))
            load_v(4)
            load_v(5)
            kt_q.append(load_kt_quarter(3))
            load_v(6)
            load_v(7)
            qt_rest = [load_qt_chunk(c) for c in (1, 2, 3)]

            def v_tile(mt):
                # returns the (low, high) D-half slices of the v row-tile mt
                g, tg = mt // VPG, mt % VPG
                if g == 0:
                    vt, tg = v_sb[0][tg // 2], tg % 2
                else:
                    vt = v_sb[g]
                return vt[:, tg, 0:D_HALF], vt[:, tg, D_HALF:D]

            # ---- PE warm-up: the HAM clock gate defaults to half rate and
            # only releases after ~3.4us of sustained PE activity. Dummy
            # matmuls (no DMA dependency) fill the otherwise-idle DMA lead-in
            # so the real work starts at full clock.
            junk = singles.tile([128, 512], f16, tag="junk")
            nc.vector.memset(junk, 0.0)
            wps = pscore.tile([128, 512], f32, tag="scores", name="warm")
            for _ in range(40):
                nc.tensor.matmul(wps, lhsT=junk[:, 0:128], rhs=junk,
                                 start=True, stop=True, skip_group_check=True)

            # ---- projection emitters ----
            # plain base-0 matmuls; the strip-1 destinations are written by
            # partition-shifted DVE copies (PSUM parts 0-31 -> SBUF 32-63)
            def u_chunk(c, qtile):
                pu = pscore.tile([R, 512], f32, tag="scores", name=f"pu{c}")
                for t in range(DT):
                    nc.tensor.matmul(pu, lhsT=wu_sb[:, t, :],
                                     rhs=qtile[:, t, :], start=(t == 0),
                                     stop=(t == DT - 1))
                nc.vector.tensor_copy(out=uT[0:R, c * 512:(c + 1) * 512],
                                      in_=pu)
                nc.vector.tensor_copy(out=uT[R:2 * R, c * 512:(c + 1) * 512],
                                      in_=pu)

            def vp_quarter(qtr):
                kt = kt_q[qtr]
                pv0 = pscore.tile([R, 4, 128], f32, tag="scores",
                                  name=f"pv0_{qtr}")
                for t in range(DT):
                    nc.tensor.matmul(pv0, lhsT=wv_sb[:, t, :],
                                     rhs=kt[:, t, 0:512], start=(t == 0),
                                     stop=(t == DT - 1))
                nc.vector.tensor_copy(
                    out=vpT[0:R, qtr * 4:(qtr + 1) * 4, :], in_=pv0)
                pv1 = pscore.tile([R, 4, 128], f32, tag="scores",
                                  name=f"pv1_{qtr}")
                for t in range(DT):
                    nc.tensor.matmul(pv1, lhsT=wv_sb[:, t, :],
                                     rhs=kt[:, t, 512:1024], start=(t == 0),
                                     stop=(t == DT - 1))
                nc.vector.tensor_copy(
                    out=vpT[R:2 * R, qtr * 4:(qtr + 1) * 4, :], in_=pv1)

            vp_quarter(0)
            u_chunk(0, qt0)

            # ---- main loop: flash-style scores/softmax/AV ----
            # software-pipelined: scores/exp for m-tile mt+1 are issued before
            # the AV matmuls of m-tile mt, so ScalarE exp latency hides under
            # the previous tile's AV work on the PE. The pipeline never crosses
            # a segment boundary (chunk 0's segments are separated by vp
            # emissions whose matmuls must precede the dependent scores).
            def scores_quad(ch, cp):
                # a COUPLE of m-tile pairs (pairs 2cp, 2cp+1): four row-tiled
                # K=32 matmuls. Different row strips run concurrently on
                # different 32-row strips of the PE array; each strip gets its
                # OWN psum bank (mixed-strip writes into one bank crash the
                # exec unit), with the two same-strip pair outputs sharing
                # that bank at column offsets 0/256.
                ps = [pscore.tile([128, 2 * N_CHUNK], f32, tag="scores",
                                  name=f"ps{g}_{ch}_{cp}") for g in range(2)]
                for hp in range(2):
                    p = 2 * cp + hp
                    for g in range(2):
                        nc.tensor.matmul(
                            ps[g][:, hp * N_CHUNK:(hp + 1) * N_CHUNK],
                            lhsT=vpT[g * R:(g + 1) * R, p, :],
                            rhs=uT[g * R:(g + 1) * R,
                                   ch * N_CHUNK:(ch + 1) * N_CHUNK],
                            start=True, stop=True, skip_group_check=True)
                exs = [expp.tile([128, 2 * N_CHUNK], f16, tag="ex",
                                 name=f"ex{ch}_{2 * cp + hp}")
                       for hp in range(2)]
                # g outer: bank ps[0] is fully consumed by the first two
                # ACTIVATEs, so its pool slot frees one ACT earlier for the
                # next couple's scores matmuls (pscore has only 2 bufs)
                for g in range(2):
                    for hp in range(2):
                        nc.scalar.activation(
                            out=exs[hp][:, g * N_CHUNK:(g + 1) * N_CHUNK],
                            in_=ps[g][:, hp * N_CHUNK:(hp + 1) * N_CHUNK],
                            func=EXP, scale=RSCALE)
                return exs

            # ---- flattened global schedule ----
            # scores are emitted in bursts of 3 (the pscore pool depth), two
            # groups ahead of the AV consumption: a dense LDW/MM burst
            # pipelines the vpT weight loads, instead of paying an unhidden
            # weight-load serialization against the AV stream on every m-tile
            # (the 32-row scores weights conflict with in-flight full-array
            # matmuls, so their loads cannot be hoisted by the PE reorder
            # window). The schedule runs across chunk boundaries so the next
            # chunk's scores pipeline is primed before the previous chunk
            # drains. Projection work (vpT quarters, uT chunks) is emitted
            # right before the first burst that consumes it.
            inserts = {
                (0, 2): lambda: vp_quarter(1),
                (0, 4): lambda: vp_quarter(2),
                (0, 6): lambda: vp_quarter(3),
                (1, 0): lambda: u_chunk(1, qt_rest[0]),
                (2, 0): lambda: u_chunk(2, qt_rest[1]),
                (3, 0): lambda: u_chunk(3, qt_rest[2]),
            }
            seq = [(ch, cp) for ch in range(NCH) for cp in range(MT // 4)]
            accs_by_ch = {}
            sums_by_ch = {}

            def get_acc(ch):
                # both sums accumulators share one bank: start=True clears
                # has_written bank-wide, so ONLY sums[0]'s first matmul
                # carries start=True; the cleared has_written makes sums[1]'s
                # first start=False matmul overwrite rather than accumulate
                if ch not in accs_by_ch:
                    accs_by_ch[ch] = [
                        pacc.tile([128, D_HALF], f32, tag="acc",
                                  name=f"acc{ch}_{i}") for i in range(4)]
                    st = psums.tile([128, 4], f32, tag="sums", name=f"sum{ch}")
                    sums_by_ch[ch] = [st[:, 0:2], st[:, 2:4]]
                return accs_by_ch[ch], sums_by_ch[ch]

            def norm_out(ch):
                accs, sums = accs_by_ch[ch], sums_by_ch[ch]
                for j in range(2):
                    rc = rpool.tile([128, 1], f32, tag="rc", name=f"rc{ch}_{j}")
                    nc.vector.reciprocal(rc, sums[j][:, 0:1])
                    ob = outp.tile([128, D], f32, tag="ob", name=f"ob{ch}_{j}")
                    nc.vector.tensor_scalar_mul(ob[:, 0:D_HALF], accs[2 * j], rc)
                    nc.vector.tensor_scalar_mul(ob[:, D_HALF:D],
                                                accs[2 * j + 1], rc)
                    row = ch * N_CHUNK + j * 128
                    nc.sync.dma_start(out=o[row:row + 128, :], in_=ob)

            ex_q = []

            def emit_couple(items):
                for ch, cp in items:
                    if (ch, cp) in inserts:
                        inserts.pop((ch, cp))()
                    ex_q.extend(scores_quad(ch, cp))

            emit_couple(seq[0:1])
            for i in range(len(seq)):
                emit_couple(seq[i + 1:i + 2])
                ch, cp = seq[i]
                accs, sums = get_acc(ch)
                for hp in range(2):
                    ex = ex_q.pop(0)
                    for sub in range(2):
                        mt = 4 * cp + 2 * hp + sub
                        vlo, vhi = v_tile(mt)
                        first, last = (mt == 0), (mt == MT - 1)
                        for j in range(2):
                            lhs = ex[:, sub * N_CHUNK + j * 128:
                                     sub * N_CHUNK + (j + 1) * 128]
                            nc.tensor.matmul(accs[2 * j], lhsT=lhs, rhs=vlo,
                                             start=first, stop=last)
                            nc.tensor.matmul(accs[2 * j + 1], lhsT=lhs,
                                             rhs=vhi, start=first, stop=last)
                            nc.tensor.matmul(sums[j], lhsT=lhs, rhs=ones,
                                             start=(first and j == 0),
                                             stop=last, skip_group_check=True)
                if cp == MT // 4 - 1:
                    norm_out(ch)

    nc.finalize()
    return nc


def kernel(q, k, v, Wu, Wv):
    global LAST_RESULT
    from concourse import bass_utils

    nc = _build()

    # permute kT columns: within each 1024-col quarter, even m-tiles' columns
    # first, then odd ones (see load_kt_quarter)
    def perm_kt(kb):
        kTb = kb.T.reshape(D, N // 1024, 4, 2, 128).swapaxes(2, 3)
        return np.ascontiguousarray(kTb.reshape(D, N)).astype(np.float16)

    kTs = [perm_kt(k[b]) for b in range(B)]
    vs = [np.ascontiguousarray(v[b]).astype(np.float16) for b in range(B)]
    wu16 = np.ascontiguousarray(Wu).astype(np.float16)
    wv16 = np.ascontiguousarray(Wv).astype(np.float16)
    in_maps = []
    for core in range(8):
        b, h = core // 2, core % 2
        in_maps.append({
            "qT": np.ascontiguousarray(
                q[b].T[:, h * NLOC:(h + 1) * NLOC]).astype(np.float16),
            "kT": kTs[b],
            "v": vs[b],
            "wu": wu16,
            "wv": wv16,
        })

    res = bass_utils.run_bass_kernel_spmd(nc, in_maps, core_ids=list(range(8)))
    LAST_RESULT = res

    out = np.empty((B, N, D), dtype=np.float32)
    for core in range(8):
        b, h = core // 2, core % 2
        out[b, h * NLOC:(h + 1) * NLOC, :] = res.results[core]["o"]
    return out


# revision 30
# speedup vs baseline: 1.0001x; 1.0001x over previous
"""Low-rank attention kernel for Trainium2, distributed over 8 NeuronCores.

Math (per batch b):
    u  = q @ Wu            [N, R]
    vp = k @ Wv            [N, R]
    S  = u @ vp.T / sqrt(R)
    out = softmax(S) @ v   [N, D]

Shapes: B=4, N=4096, D=1024, R=32.

Sharding: data-parallel over batch x row-halves -> 8 shards. Core c handles
batch b = c // 2, rows [h*2048, (h+1)*2048) with h = c % 2. Each core gets its
q-shard and the full k/v for its batch. q/k are fed pre-transposed ([D, n]
layout, fp16) so every matmul contraction lands on the partition axis with no
on-device transposes.

Per-core device kernel (all matmuls fp16: 1 col/cycle full PE rate):
  1. uT[2R, 2048] = sum_d Wu[d, :].T qT[d, :]   (K=128 d-tiles, PSUM accum,
     replicated onto partition strips 0-31 / 32-63 by shifted DVE copies)
     vpT[2R, 16, 128] = sum_d Wv[d, :].T kT[d, :]  (even m-tiles on strip 0,
     odd on strip 1; kT arrives host-permuted so both are plain slices)
  2. flash-style main loop over n-chunks of 256 rows, in COUPLES of two
     m-tile pairs:
       scores: 4 row-tiled K=32 matmuls; the two strips run concurrently on
       different 32-row strips of the PE array; each strip owns its PSUM
       bank (mixed-strip writes into one bank crash the exec unit), with the
       same-strip pair outputs sharing the bank at column offsets 0/256
       expT = Exp(scoresT / sqrt(R))                   (ScalarE, PSUM->SBUF)
       out_acc[n128, d512] += expT_tile.T @ v_tile    (PSUM accum over m)
       sum_acc[n128, 1]    += expT_tile.T @ ones
     out = out_acc * (1 / sum_acc)   (softmax normalization folded at the end)

Scheduling: one flat software-pipelined schedule across all chunks — the
scores/exp of couple k+1 are emitted before the AV matmuls of couple k, so
the ScalarE exp latency and the scores weight loads hide under AV work, and
chunk boundaries never drain the pipeline. The vpT quarters and uT chunks
are emitted right before the first couple that consumes them, overlapping
the kT/qT DMA with chunk-0/1-3 AV work. Dummy warm-up matmuls during the
DMA lead-in hold the PE's HAM clock gate at full rate before real work
arrives.
"""

import numpy as np

B, N, D, R = 4, 4096, 1024, 32
NLOC = N // 2            # rows per core
RSCALE = float(1.0 / np.sqrt(np.float32(R)))

N_CHUNK = 256            # rows of scores computed per PSUM round
D_HALF = 512             # PSUM bank width in fp32

LAST_RESULT = None       # test.py reads exec_time_ns etc. from here


def _build():
    from concourse import bacc, mybir
    from concourse.tile import TileContext

    f32 = mybir.dt.float32
    f16 = mybir.dt.float16
    EXP = mybir.ActivationFunctionType.Exp

    nc = bacc.Bacc("TRN2", target_bir_lowering=False)

    qT = nc.dram_tensor("qT", [D, NLOC], f16, kind="ExternalInput")
    kT = nc.dram_tensor("kT", [D, N], f16, kind="ExternalInput")
    v = nc.dram_tensor("v", [N, D], f16, kind="ExternalInput")
    wu = nc.dram_tensor("wu", [D, R], f16, kind="ExternalInput")
    wv = nc.dram_tensor("wv", [D, R], f16, kind="ExternalInput")
    o = nc.dram_tensor("o", [NLOC, D], f32, kind="ExternalOutput")

    DT = D // 128         # 8 d-tiles
    UC = NLOC // 512      # 4 uT column-chunks
    MQ = N // 1024        # 4 kT quarters
    NCH = NLOC // N_CHUNK  # 8 main-loop chunks
    MT = N // 128         # 32 m tiles
    VG = 8                # v row-groups of 512
    VPG = N // VG // 128  # 4 m-tiles per v group

    with TileContext(nc) as tc:
        with tc.tile_pool(name="singles", bufs=1) as singles, \
             tc.tile_pool(name="ktp", bufs=3) as ktp, \
             tc.tile_pool(name="qtp", bufs=4) as qtp, \
             tc.tile_pool(name="vpool", bufs=VG - 1) as vpool, \
             tc.tile_pool(name="expp", bufs=6) as expp, \
             tc.tile_pool(name="outp", bufs=3) as outp, \
             tc.tile_pool(name="rpool", bufs=4) as rpool, \
             tc.tile_pool(name="pacc", bufs=5, space="PSUM") as pacc, \
             tc.tile_pool(name="pscore", bufs=2, space="PSUM") as pscore, \
             tc.tile_pool(name="psums", bufs=1, space="PSUM") as psums:

            # ---- constants / projection weights ----
            wu_sb = singles.tile([128, DT, R], f16, tag="wu")
            nc.sync.dma_start(out=wu_sb, in_=wu.rearrange("(t p) r -> p t r", p=128))
            wv_sb = singles.tile([128, DT, R], f16, tag="wv")
            nc.sync.dma_start(out=wv_sb, in_=wv.rearrange("(t p) r -> p t r", p=128))
            ones = singles.tile([128, 2], f16, tag="ones")
            nc.vector.memset(ones, 1.0)

            # uT is replicated on partition groups 0-31 / 32-63 and vpT is
            # distributed even/odd-m-tile across them, so the scores matmuls
            # for an m-tile PAIR run concurrently as two row-tiled (K=32)
            # matmuls on different 32-row strips of the PE array.
            uT = singles.tile([2 * R, NLOC], f16, tag="uT")
            vpT = singles.tile([2 * R, MT // 2, 128], f16, tag="vpT")

            # ---- DMA emission, in order of need (one trigger per load:
            # DMA trigger instructions serialize at ~650ns on the Sync queue)
            def load_qt_chunk(c):
                tile = qtp.tile([128, DT, 512], f16, tag="qt", name=f"qt{c}")
                nc.sync.dma_start(
                    out=tile,
                    in_=qT[:, c * 512:(c + 1) * 512].rearrange(
                        "(t p) c -> p t c", p=128))
                return tile

            def load_kt_quarter(q):
                # kT arrives host-permuted: within each 1024-col quarter, the
                # even m-tiles' 512 columns come first, then the odd ones, so
                # the vp projection's even/odd moving operands are plain
                # slices
                tile = ktp.tile([128, DT, 1024], f16, tag="kt", name=f"kt{q}")
                nc.sync.dma_start(
                    out=tile,
                    in_=kT[:, q * 1024:(q + 1) * 1024].rearrange(
                        "(t p) c -> p t c", p=128))
                return tile

            v_sb = [None] * VG

            def load_v(g):
                vt = vpool.tile([128, VPG, D], f16, tag="v", name=f"v{g}")
                nc.sync.dma_start(
                    out=vt, in_=v[g * 512:(g + 1) * 512, :].rearrange(
                        "(t p) d -> p t d", p=128))
                v_sb[g] = vt

            # group 0 is split in half so the first AV matmul only waits on
            # 0.5MB of v
            def load_v0():
                va = vpool.tile([128, 2, D], f16, tag="v0a", name="v0a", bufs=1)
                nc.sync.dma_start(
                    out=va, in_=v[0:256, :].rearrange("(t p) d -> p t d", p=128))
                vb = vpool.tile([128, 2, D], f16, tag="v0b", name="v0b", bufs=1)
                nc.sync.dma_start(
                    out=vb, in_=v[256:512, :].rearrange("(t p) d -> p t d", p=128))
                v_sb[0] = (va, vb)

            kt_q = [load_kt_quarter(0)]
            qt0 = load_qt_chunk(0)
            load_v0()
            load_v(1)
            kt_q.append(load_kt_quarter(1))
            load_v(2)
            load_v(3)
            kt_q.append(load_kt_quarter(2))
            load_v(4)
            load_v(5)
            kt_q.append(load_kt_quarter(3))
            load_v(6)
            load_v(7)
            qt_rest = [load_qt_chunk(c) for c in (1, 2, 3)]

            def v_tile(mt):
                # returns the (low, high) D-half slices of the v row-tile mt
                g, tg = mt // VPG, mt % VPG
                if g == 0:
                    vt, tg = v_sb[0][tg // 2], tg % 2
                else:
                    vt = v_sb[g]
                return vt[:, tg, 0:D_HALF], vt[:, tg, D_HALF:D]

            # ---- PE warm-up: the HAM clock gate defaults to half rate and
            # only releases after ~3.4us of sustained PE activity. Dummy
            # matmuls (no DMA dependency) fill the otherwise-idle DMA lead-in
            # so the real work starts at full clock.
            junk = singles.tile([128, 512], f16, tag="junk")
            nc.vector.memset(junk, 0.0)
            wps = pscore.tile([128, 512], f32, tag="scores", name="warm")
            for _ in range(40):
                nc.tensor.matmul(wps, lhsT=junk[:, 0:128], rhs=junk,
                                 start=True, stop=True, skip_group_check=True)

            # ---- projection emitters ----
            # plain base-0 matmuls; the strip-1 destinations are written by
            # partition-shifted DVE copies (PSUM parts 0-31 -> SBUF 32-63)
            def u_chunk(c, qtile):
                pu = pscore.tile([R, 512], f32, tag="scores", name=f"pu{c}")
                for t in range(DT):
                    nc.tensor.matmul(pu, lhsT=wu_sb[:, t, :],
                                     rhs=qtile[:, t, :], start=(t == 0),
                                     stop=(t == DT - 1))
                nc.vector.tensor_copy(out=uT[0:R, c * 512:(c + 1) * 512],
                                      in_=pu)
                nc.vector.tensor_copy(out=uT[R:2 * R, c * 512:(c + 1) * 512],
                                      in_=pu)

            def vp_quarter(qtr):
                kt = kt_q[qtr]
                pv0 = pscore.tile([R, 4, 128], f32, tag="scores",
                                  name=f"pv0_{qtr}")
                for t in range(DT):
                    nc.tensor.matmul(pv0, lhsT=wv_sb[:, t, :],
                                     rhs=kt[:, t, 0:512], start=(t == 0),
                                     stop=(t == DT - 1))
                nc.vector.tensor_copy(
                    out=vpT[0:R, qtr * 4:(qtr + 1) * 4, :], in_=pv0)
                pv1 = pscore.tile([R, 4, 128], f32, tag="scores",
                                  name=f"pv1_{qtr}")
                for t in range(DT):
                    nc.tensor.matmul(pv1, lhsT=wv_sb[:, t, :],
                                     rhs=kt[:, t, 512:1024], start=(t == 0),
                                     stop=(t == DT - 1))
                nc.vector.tensor_copy(
                    out=vpT[R:2 * R, qtr * 4:(qtr + 1) * 4, :], in_=pv1)

            vp_quarter(0)
            u_chunk(0, qt0)

            # ---- main loop: flash-style scores/softmax/AV ----
            # software-pipelined: scores/exp for m-tile mt+1 are issued before
            # the AV matmuls of m-tile mt, so ScalarE exp latency hides under
            # the previous tile's AV work on the PE. The pipeline never crosses
            # a segment boundary (chunk 0's segments are separated by vp
            # emissions whose matmuls must precede the dependent scores).
            def scores_quad(ch, cp):
                # a COUPLE of m-tile pairs (pairs 2cp, 2cp+1): four row-tiled
                # K=32 matmuls. Different row strips run concurrently on
                # different 32-row strips of the PE array; each strip gets its
                # OWN psum bank (mixed-strip writes into one bank crash the
                # exec unit), with the two same-strip pair outputs sharing
                # that bank at column offsets 0/256.
                ps = [pscore.tile([128, 2 * N_CHUNK], f32, tag="scores",
                                  name=f"ps{g}_{ch}_{cp}") for g in range(2)]
                for hp in range(2):
                    p = 2 * cp + hp
                    for g in range(2):
                        nc.tensor.matmul(
                            ps[g][:, hp * N_CHUNK:(hp + 1) * N_CHUNK],
                            lhsT=vpT[g * R:(g + 1) * R, p, :],
                            rhs=uT[g * R:(g + 1) * R,
                                   ch * N_CHUNK:(ch + 1) * N_CHUNK],
                            start=True, stop=True, skip_group_check=True)
                exs = []
                for hp in range(2):
                    ex = expp.tile([128, 2 * N_CHUNK], f16, tag="ex",
                                   name=f"ex{ch}_{2 * cp + hp}")
                    for g in range(2):
                        nc.scalar.activation(
                            out=ex[:, g * N_CHUNK:(g + 1) * N_CHUNK],
                            in_=ps[g][:, hp * N_CHUNK:(hp + 1) * N_CHUNK],
                            func=EXP, scale=RSCALE)
                    exs.append(ex)
                return exs

            # ---- flattened global schedule ----
            # scores are emitted in bursts of 3 (the pscore pool depth), two
            # groups ahead of the AV consumption: a dense LDW/MM burst
            # pipelines the vpT weight loads, instead of paying an unhidden
            # weight-load serialization against the AV stream on every m-tile
            # (the 32-row scores weights conflict with in-flight full-array
            # matmuls, so their loads cannot be hoisted by the PE reorder
            # window). The schedule runs across chunk boundaries so the next
            # chunk's scores pipeline is primed before the previous chunk
            # drains. Projection work (vpT quarters, uT chunks) is emitted
            # right before the first burst that consumes it.
            inserts = {
                (0, 2): lambda: vp_quarter(1),
                (0, 4): lambda: vp_quarter(2),
                (0, 6): lambda: vp_quarter(3),
                (1, 0): lambda: u_chunk(1, qt_rest[0]),
                (2, 0): lambda: u_chunk(2, qt_rest[1]),
                (3, 0): lambda: u_chunk(3, qt_rest[2]),
            }
            seq = [(ch, cp) for ch in range(NCH) for cp in range(MT // 4)]
            accs_by_ch = {}
            sums_by_ch = {}

            def get_acc(ch):
                # both sums accumulators share one bank: start=True clears
                # has_written bank-wide, so ONLY sums[0]'s first matmul
                # carries start=True; the cleared has_written makes sums[1]'s
                # first start=False matmul overwrite rather than accumulate
                if ch not in accs_by_ch:
                    accs_by_ch[ch] = [
                        pacc.tile([128, D_HALF], f32, tag="acc",
                                  name=f"acc{ch}_{i}") for i in range(4)]
                    st = psums.tile([128, 4], f32, tag="sums", name=f"sum{ch}")
                    sums_by_ch[ch] = [st[:, 0:2], st[:, 2:4]]
                return accs_by_ch[ch], sums_by_ch[ch]

            def norm_out(ch):
                accs, sums = accs_by_ch[ch], sums_by_ch[ch]
                for j in range(2):
                    rc = rpool.tile([128, 1], f32, tag="rc", name=f"rc{ch}_{j}")
                    nc.vector.reciprocal(rc, sums[j][:, 0:1])
                    ob = outp.tile([128, D], f32, tag="ob", name=f"ob{ch}_{j}")
                    nc.vector.tensor_scalar_mul(ob[:, 0:D_HALF], accs[2 * j], rc)
                    nc.vector.tensor_scalar_mul(ob[:, D_HALF:D],
                                                accs[2 * j + 1], rc)
                    row = ch * N_CHUNK + j * 128
                    nc.sync.dma_start(out=o[row:row + 128, :], in_=ob)

            ex_q = []

            def emit_couple(items):
                for ch, cp in items:
                    if (ch, cp) in inserts:
                        inserts.pop((ch, cp))()
                    ex_q.extend(scores_quad(ch, cp))

            emit_couple(seq[0:1])
            for i in range(len(seq)):
                emit_couple(seq[i + 1:i + 2])
                ch, cp = seq[i]
                accs, sums = get_acc(ch)
                for hp in range(2):
                    ex = ex_q.pop(0)
                    for sub in range(2):
                        mt = 4 * cp + 2 * hp + sub
                        vlo, vhi = v_tile(mt)
                        first, last = (mt == 0), (mt == MT - 1)
                        for j in range(2):
                            lhs = ex[:, sub * N_CHUNK + j * 128:
                                     sub * N_CHUNK + (j + 1) * 128]
                            nc.tensor.matmul(accs[2 * j], lhsT=lhs, rhs=vlo,
                                             start=first, stop=last)
                            nc.tensor.matmul(accs[2 * j + 1], lhsT=lhs,
                                             rhs=vhi, start=first, stop=last)
                            nc.tensor.matmul(sums[j], lhsT=lhs, rhs=ones,
                                             start=(first and j == 0),
                                             stop=last, skip_group_check=True)
                if cp == MT // 4 - 1:
                    norm_out(ch)

    nc.finalize()
    return nc


def kernel(q, k, v, Wu, Wv):
    global LAST_RESULT
    from concourse import bass_utils

    nc = _build()

    # permute kT columns: within each 1024-col quarter, even m-tiles' columns
    # first, then odd ones (see load_kt_quarter)
    def perm_kt(kb):
        kTb = kb.T.reshape(D, N // 1024, 4, 2, 128).swapaxes(2, 3)
        return np.ascontiguousarray(kTb.reshape(D, N)).astype(np.float16)

    kTs = [perm_kt(k[b]) for b in range(B)]
    vs = [np.ascontiguousarray(v[b]).astype(np.float16) for b in range(B)]
    wu16 = np.ascontiguousarray(Wu).astype(np.float16)
    wv16 = np.ascontiguousarray(Wv).astype(np.float16)
    in_maps = []
    for core in range(8):
        b, h = core // 2, core % 2
        in_maps.append({
            "qT": np.ascontiguousarray(
                q[b].T[:, h * NLOC:(h + 1) * NLOC]).astype(np.float16),
            "kT": kTs[b],
            "v": vs[b],
            "wu": wu16,
            "wv": wv16,
        })

    res = bass_utils.run_bass_kernel_spmd(nc, in_maps, core_ids=list(range(8)))
    LAST_RESULT = res

    out = np.empty((B, N, D), dtype=np.float32)
    for core in range(8):
        b, h = core // 2, core % 2
        out[b, h * NLOC:(h + 1) * NLOC, :] = res.results[core]["o"]
    return out


# revision 36
# speedup vs baseline: 1.0051x; 1.0050x over previous
"""Low-rank attention kernel for Trainium2, distributed over 8 NeuronCores.

Math (per batch b):
    u  = q @ Wu            [N, R]
    vp = k @ Wv            [N, R]
    S  = u @ vp.T / sqrt(R)
    out = softmax(S) @ v   [N, D]

Shapes: B=4, N=4096, D=1024, R=32.

Sharding: data-parallel over batch x row-halves -> 8 shards. Core c handles
batch b = c // 2, rows [h*2048, (h+1)*2048) with h = c % 2. Each core gets its
q-shard and the full k/v for its batch. q/k are fed pre-transposed ([D, n]
layout, fp16) so every matmul contraction lands on the partition axis with no
on-device transposes.

Per-core device kernel (all matmuls fp16: 1 col/cycle full PE rate):
  1. uT[2R, 2048] = sum_d Wu[d, :].T qT[d, :]   (K=128 d-tiles, PSUM accum,
     replicated onto partition strips 0-31 / 32-63 by shifted DVE copies)
     vpT[2R, 16, 128] = sum_d Wv[d, :].T kT[d, :]  (even m-tiles on strip 0,
     odd on strip 1; kT arrives host-permuted so both are plain slices)
  2. flash-style main loop over n-chunks of 256 rows, in COUPLES of two
     m-tile pairs:
       scores: 4 row-tiled K=32 matmuls; the two strips run concurrently on
       different 32-row strips of the PE array; each strip owns its PSUM
       bank (mixed-strip writes into one bank crash the exec unit), with the
       same-strip pair outputs sharing the bank at column offsets 0/256
       expT = Exp(scoresT / sqrt(R))                   (ScalarE, PSUM->SBUF)
       out_acc[n128, d512] += expT_tile.T @ v_tile    (PSUM accum over m)
       sum_acc[n128, 1]    += expT_tile.T @ ones
     out = out_acc * (1 / sum_acc)   (softmax normalization folded at the end)

Scheduling: one flat software-pipelined schedule across all chunks — the
scores/exp of couple k+1 are emitted before the AV matmuls of couple k, so
the ScalarE exp latency and the scores weight loads hide under AV work, and
chunk boundaries never drain the pipeline. The vpT quarters and uT chunks
are emitted right before the first couple that consumes them, overlapping
the kT/qT DMA with chunk-0/1-3 AV work. Dummy warm-up matmuls during the
DMA lead-in hold the PE's HAM clock gate at full rate before real work
arrives.
"""

import numpy as np

B, N, D, R = 4, 4096, 1024, 32
NLOC = N // 2            # rows per core
RSCALE = float(1.0 / np.sqrt(np.float32(R)))

N_CHUNK = 256            # rows of scores computed per PSUM round
D_HALF = 512             # PSUM bank width in fp32

LAST_RESULT = None       # test.py reads exec_time_ns etc. from here


def _build():
    from concourse import bacc, mybir
    from concourse.tile import TileContext

    f32 = mybir.dt.float32
    f16 = mybir.dt.float16
    EXP = mybir.ActivationFunctionType.Exp

    nc = bacc.Bacc("TRN2", target_bir_lowering=False)

    qT = nc.dram_tensor("qT", [D, NLOC], f16, kind="ExternalInput")
    kT = nc.dram_tensor("kT", [D, N], f16, kind="ExternalInput")
    v = nc.dram_tensor("v", [N, D], f16, kind="ExternalInput")
    wu = nc.dram_tensor("wu", [D, R], f16, kind="ExternalInput")
    wv = nc.dram_tensor("wv", [D, R], f16, kind="ExternalInput")
    o = nc.dram_tensor("o", [NLOC, D], f32, kind="ExternalOutput")

    DT = D // 128         # 8 d-tiles
    UC = NLOC // 512      # 4 uT column-chunks
    MQ = N // 1024        # 4 kT quarters
    NCH = NLOC // N_CHUNK  # 8 main-loop chunks
    MT = N // 128         # 32 m tiles
    VG = 8                # v row-groups of 512
    VPG = N // VG // 128  # 4 m-tiles per v group

    with TileContext(nc) as tc:
        with tc.tile_pool(name="singles", bufs=1) as singles, \
             tc.tile_pool(name="ktp", bufs=4) as ktp, \
             tc.tile_pool(name="qtp", bufs=4) as qtp, \
             tc.tile_pool(name="vpool", bufs=VG - 1) as vpool, \
             tc.tile_pool(name="expp", bufs=6) as expp, \
             tc.tile_pool(name="outp", bufs=3) as outp, \
             tc.tile_pool(name="rpool", bufs=4) as rpool, \
             tc.tile_pool(name="pacc", bufs=5, space="PSUM") as pacc, \
             tc.tile_pool(name="pscore", bufs=2, space="PSUM") as pscore, \
             tc.tile_pool(name="psums", bufs=1, space="PSUM") as psums:

            # ---- constants / projection weights ----
            wu_sb = singles.tile([128, DT, R], f16, tag="wu")
            nc.sync.dma_start(out=wu_sb, in_=wu.rearrange("(t p) r -> p t r", p=128))
            wv_sb = singles.tile([128, DT, R], f16, tag="wv")
            nc.sync.dma_start(out=wv_sb, in_=wv.rearrange("(t p) r -> p t r", p=128))
            ones = singles.tile([128, 2], f16, tag="ones")
            nc.vector.memset(ones, 1.0)

            # uT is replicated on partition groups 0-31 / 32-63 and vpT is
            # distributed even/odd-m-tile across them, so the scores matmuls
            # for an m-tile PAIR run concurrently as two row-tiled (K=32)
            # matmuls on different 32-row strips of the PE array.
            uT = singles.tile([2 * R, NLOC], f16, tag="uT")
            vpT = singles.tile([2 * R, MT // 2, 128], f16, tag="vpT")

            # ---- DMA emission, in order of need (one trigger per load:
            # DMA trigger instructions serialize at ~650ns on the Sync queue)
            def load_qt_chunk(c):
                # two d-halves, so the u projection can start on the first
                # half while the second streams in
                halves = []
                for h in range(2):
                    tile = qtp.tile([128, DT // 2, 512], f16, tag="qt",
                                    name=f"qt{c}_{h}")
                    d0 = h * (D // 2)
                    nc.sync.dma_start(
                        out=tile,
                        in_=qT[d0:d0 + D // 2,
                               c * 512:(c + 1) * 512].rearrange(
                            "(t p) c -> p t c", p=128))
                    halves.append(tile)
                return halves

            def load_kt_quarter(q):
                # kT arrives host-permuted: within each 1024-col quarter, the
                # even m-tiles' 512 columns come first, then the odd ones, so
                # the vp projection's even/odd moving operands are plain
                # slices. Loaded as two d-halves so the first projection
                # matmuls start after 1MB instead of 2MB.
                halves = []
                for h in range(2):
                    tile = ktp.tile([128, DT // 2, 1024], f16, tag="kt",
                                    name=f"kt{q}_{h}")
                    d0 = h * (D // 2)
                    nc.sync.dma_start(
                        out=tile,
                        in_=kT[d0:d0 + D // 2,
                               q * 1024:(q + 1) * 1024].rearrange(
                            "(t p) c -> p t c", p=128))
                    halves.append(tile)
                return halves

            v_sb = [None] * VG

            def load_v(g):
                vt = vpool.tile([128, VPG, D], f16, tag="v", name=f"v{g}")
                nc.sync.dma_start(
                    out=vt, in_=v[g * 512:(g + 1) * 512, :].rearrange(
                        "(t p) d -> p t d", p=128))
                v_sb[g] = vt

            # group 0 is split in half so the first AV matmul only waits on
            # 0.5MB of v
            def load_v0():
                va = vpool.tile([128, 2, D], f16, tag="v0a", name="v0a", bufs=1)
                nc.sync.dma_start(
                    out=va, in_=v[0:256, :].rearrange("(t p) d -> p t d", p=128))
                vb = vpool.tile([128, 2, D], f16, tag="v0b", name="v0b", bufs=1)
                nc.sync.dma_start(
                    out=vb, in_=v[256:512, :].rearrange("(t p) d -> p t d", p=128))
                v_sb[0] = (va, vb)

            kt_q = [load_kt_quarter(0)]
            qt0 = load_qt_chunk(0)
            load_v0()
            load_v(1)
            kt_q.append(load_kt_quarter(1))
            load_v(2)
            load_v(3)
            kt_q.append(load_kt_quarter(2))
            load_v(4)
            load_v(5)
            kt_q.append(load_kt_quarter(3))
            load_v(6)
            load_v(7)
            qt_rest = [load_qt_chunk(c) for c in (1, 2, 3)]

            def v_tile(mt):
                # returns the (low, high) D-half slices of the v row-tile mt
                g, tg = mt // VPG, mt % VPG
                if g == 0:
                    vt, tg = v_sb[0][tg // 2], tg % 2
                else:
                    vt = v_sb[g]
                return vt[:, tg, 0:D_HALF], vt[:, tg, D_HALF:D]

            # ---- PE warm-up: the HAM clock gate defaults to half rate and
            # only releases after ~3.4us of sustained PE activity. Dummy
            # matmuls (no DMA dependency) fill the otherwise-idle DMA lead-in
            # so the real work starts at full clock.
            junk = singles.tile([128, 512], f16, tag="junk")
            nc.vector.memset(junk, 0.0)
            wps = pscore.tile([128, 512], f32, tag="scores", name="warm")
            for _ in range(40):
                nc.tensor.matmul(wps, lhsT=junk[:, 0:128], rhs=junk,
                                 start=True, stop=True, skip_group_check=True)

            # ---- projection emitters ----
            # plain base-0 matmuls; the strip-1 destinations are written by
            # partition-shifted DVE copies (PSUM parts 0-31 -> SBUF 32-63)
            def u_chunk(c, qtile):
                qa, qb = qtile
                pu = pscore.tile([R, 512], f32, tag="scores", name=f"pu{c}")
                for t in range(DT):
                    qh = qa if t < DT // 2 else qb
                    nc.tensor.matmul(pu, lhsT=wu_sb[:, t, :],
                                     rhs=qh[:, t % (DT // 2), :],
                                     start=(t == 0), stop=(t == DT - 1))
                nc.vector.tensor_copy(out=uT[0:R, c * 512:(c + 1) * 512],
                                      in_=pu)
                nc.vector.tensor_copy(out=uT[R:2 * R, c * 512:(c + 1) * 512],
                                      in_=pu)

            def vp_quarter(qtr):
                kta, ktb = kt_q[qtr]
                pv0 = pscore.tile([R, 4, 128], f32, tag="scores",
                                  name=f"pv0_{qtr}")
                for t in range(DT):
                    kt = kta if t < DT // 2 else ktb
                    nc.tensor.matmul(pv0, lhsT=wv_sb[:, t, :],
                                     rhs=kt[:, t % (DT // 2), 0:512],
                                     start=(t == 0), stop=(t == DT - 1))
                nc.vector.tensor_copy(
                    out=vpT[0:R, qtr * 4:(qtr + 1) * 4, :], in_=pv0)
                pv1 = pscore.tile([R, 4, 128], f32, tag="scores",
                                  name=f"pv1_{qtr}")
                for t in range(DT):
                    kt = kta if t < DT // 2 else ktb
                    nc.tensor.matmul(pv1, lhsT=wv_sb[:, t, :],
                                     rhs=kt[:, t % (DT // 2), 512:1024],
                                     start=(t == 0), stop=(t == DT - 1))
                nc.vector.tensor_copy(
                    out=vpT[R:2 * R, qtr * 4:(qtr + 1) * 4, :], in_=pv1)

            vp_quarter(0)
            u_chunk(0, qt0)

            # ---- main loop: flash-style scores/softmax/AV ----
            # software-pipelined: scores/exp for m-tile mt+1 are issued before
            # the AV matmuls of m-tile mt, so ScalarE exp latency hides under
            # the previous tile's AV work on the PE. The pipeline never crosses
            # a segment boundary (chunk 0's segments are separated by vp
            # emissions whose matmuls must precede the dependent scores).
            def scores_quad(ch, cp):
                # a COUPLE of m-tile pairs (pairs 2cp, 2cp+1): four row-tiled
                # K=32 matmuls. Different row strips run concurrently on
                # different 32-row strips of the PE array; each strip gets its
                # OWN psum bank (mixed-strip writes into one bank crash the
                # exec unit), with the two same-strip pair outputs sharing
                # that bank at column offsets 0/256.
                ps = [pscore.tile([128, 2 * N_CHUNK], f32, tag="scores",
                                  name=f"ps{g}_{ch}_{cp}") for g in range(2)]
                for hp in range(2):
                    p = 2 * cp + hp
                    for g in range(2):
                        nc.tensor.matmul(
                            ps[g][:, hp * N_CHUNK:(hp + 1) * N_CHUNK],
                            lhsT=vpT[g * R:(g + 1) * R, p, :],
                            rhs=uT[g * R:(g + 1) * R,
                                   ch * N_CHUNK:(ch + 1) * N_CHUNK],
                            start=True, stop=True, skip_group_check=True)
                exs = []
                for hp in range(2):
                    ex = expp.tile([128, 2 * N_CHUNK], f16, tag="ex",
                                   name=f"ex{ch}_{2 * cp + hp}")
                    for g in range(2):
                        nc.scalar.activation(
                            out=ex[:, g * N_CHUNK:(g + 1) * N_CHUNK],
                            in_=ps[g][:, hp * N_CHUNK:(hp + 1) * N_CHUNK],
                            func=EXP, scale=RSCALE)
                    exs.append(ex)
                return exs

            # ---- flattened global schedule ----
            # scores are emitted in bursts of 3 (the pscore pool depth), two
            # groups ahead of the AV consumption: a dense LDW/MM burst
            # pipelines the vpT weight loads, instead of paying an unhidden
            # weight-load serialization against the AV stream on every m-tile
            # (the 32-row scores weights conflict with in-flight full-array
            # matmuls, so their loads cannot be hoisted by the PE reorder
            # window). The schedule runs across chunk boundaries so the next
            # chunk's scores pipeline is primed before the previous chunk
            # drains. Projection work (vpT quarters, uT chunks) is emitted
            # right before the first burst that consumes it.
            # u chunks go mid-chunk (couple 4), away from the busy chunk
            # boundary (norm DVE + next chunk's first couple)
            inserts = {
                (0, 2): lambda: vp_quarter(1),
                (0, 4): lambda: vp_quarter(2),
                (0, 6): lambda: vp_quarter(3),
                (1, 4): lambda: u_chunk(1, qt_rest[0]),
                (2, 4): lambda: u_chunk(2, qt_rest[1]),
                (3, 4): lambda: u_chunk(3, qt_rest[2]),
            }
            seq = [(ch, cp) for ch in range(NCH) for cp in range(MT // 4)]
            accs_by_ch = {}
            sums_by_ch = {}

            def get_acc(ch):
                # both sums accumulators share one bank: start=True clears
                # has_written bank-wide, so ONLY sums[0]'s first matmul
                # carries start=True; the cleared has_written makes sums[1]'s
                # first start=False matmul overwrite rather than accumulate
                if ch not in accs_by_ch:
                    accs_by_ch[ch] = [
                        pacc.tile([128, D_HALF], f32, tag="acc",
                                  name=f"acc{ch}_{i}") for i in range(4)]
                    st = psums.tile([128, 4], f32, tag="sums", name=f"sum{ch}")
                    sums_by_ch[ch] = [st[:, 0:2], st[:, 2:4]]
                return accs_by_ch[ch], sums_by_ch[ch]

            def norm_out(ch):
                accs, sums = accs_by_ch[ch], sums_by_ch[ch]
                for j in range(2):
                    rc = rpool.tile([128, 1], f32, tag="rc", name=f"rc{ch}_{j}")
                    nc.vector.reciprocal(rc, sums[j][:, 0:1])
                    ob = outp.tile([128, D], f32, tag="ob", name=f"ob{ch}_{j}")
                    nc.vector.tensor_scalar_mul(ob[:, 0:D_HALF], accs[2 * j], rc)
                    nc.vector.tensor_scalar_mul(ob[:, D_HALF:D],
                                                accs[2 * j + 1], rc)
                    row = ch * N_CHUNK + j * 128
                    nc.sync.dma_start(out=o[row:row + 128, :], in_=ob)

            ex_q = []

            def emit_couple(items):
                for ch, cp in items:
                    if (ch, cp) in inserts:
                        inserts.pop((ch, cp))()
                    ex_q.extend(scores_quad(ch, cp))

            emit_couple(seq[0:1])
            for i in range(len(seq)):
                emit_couple(seq[i + 1:i + 2])
                ch, cp = seq[i]
                accs, sums = get_acc(ch)
                for hp in range(2):
                    ex = ex_q.pop(0)
                    for sub in range(2):
                        mt = 4 * cp + 2 * hp + sub
                        vlo, vhi = v_tile(mt)
                        first, last = (mt == 0), (mt == MT - 1)
                        for j in range(2):
                            lhs = ex[:, sub * N_CHUNK + j * 128:
                                     sub * N_CHUNK + (j + 1) * 128]
                            nc.tensor.matmul(accs[2 * j], lhsT=lhs, rhs=vlo,
                                             start=first, stop=last)
                            nc.tensor.matmul(accs[2 * j + 1], lhsT=lhs,
                                             rhs=vhi, start=first, stop=last)
                            nc.tensor.matmul(sums[j], lhsT=lhs, rhs=ones,
                                             start=(first and j == 0),
                                             stop=last, skip_group_check=True)
                if cp == MT // 4 - 1:
                    norm_out(ch)

    nc.finalize()
    return nc


def kernel(q, k, v, Wu, Wv):
    global LAST_RESULT
    from concourse import bass_utils

    nc = _build()

    # permute kT columns: within each 1024-col quarter, even m-tiles' columns
    # first, then odd ones (see load_kt_quarter)
    def perm_kt(kb):
        kTb = kb.T.reshape(D, N // 1024, 4, 2, 128).swapaxes(2, 3)
        return np.ascontiguousarray(kTb.reshape(D, N)).astype(np.float16)

    kTs = [perm_kt(k[b]) for b in range(B)]
    vs = [np.ascontiguousarray(v[b]).astype(np.float16) for b in range(B)]
    wu16 = np.ascontiguousarray(Wu).astype(np.float16)
    wv16 = np.ascontiguousarray(Wv).astype(np.float16)
    in_maps = []
    for core in range(8):
        b, h = core // 2, core % 2
        in_maps.append({
            "qT": np.ascontiguousarray(
                q[b].T[:, h * NLOC:(h + 1) * NLOC]).astype(np.float16),
            "kT": kTs[b],
            "v": vs[b],
            "wu": wu16,
            "wv": wv16,
        })

    res = bass_utils.run_bass_kernel_spmd(nc, in_maps, core_ids=list(range(8)))
    LAST_RESULT = res

    out = np.empty((B, N, D), dtype=np.float32)
    for core in range(8):
        b, h = core // 2, core % 2
        out[b, h * NLOC:(h + 1) * NLOC, :] = res.results[core]["o"]
    return out


# revision 39
# speedup vs baseline: 1.0061x; 1.0010x over previous
"""Low-rank attention kernel for Trainium2, distributed over 8 NeuronCores.

Math (per batch b):
    u  = q @ Wu            [N, R]
    vp = k @ Wv            [N, R]
    S  = u @ vp.T / sqrt(R)
    out = softmax(S) @ v   [N, D]

Shapes: B=4, N=4096, D=1024, R=32.

Sharding: data-parallel over batch x row-halves -> 8 shards. Core c handles
batch b = c // 2, rows [h*2048, (h+1)*2048) with h = c % 2. Each core gets its
q-shard and the full k/v for its batch. q/k are fed pre-transposed ([D, n]
layout, fp16) so every matmul contraction lands on the partition axis with no
on-device transposes.

Per-core device kernel (all matmuls fp16: 1 col/cycle full PE rate):
  1. uT[2R, 2048] = sum_d Wu[d, :].T qT[d, :]   (K=128 d-tiles, PSUM accum,
     replicated onto partition strips 0-31 / 32-63 by shifted DVE copies)
     vpT[2R, 16, 128] = sum_d Wv[d, :].T kT[d, :]  (even m-tiles on strip 0,
     odd on strip 1; kT arrives host-permuted so both are plain slices)
  2. flash-style main loop over n-chunks of 256 rows, in COUPLES of two
     m-tile pairs:
       scores: 4 row-tiled K=32 matmuls; the two strips run concurrently on
       different 32-row strips of the PE array; each strip owns its PSUM
       bank (mixed-strip writes into one bank crash the exec unit), with the
       same-strip pair outputs sharing the bank at column offsets 0/256
       expT = Exp(scoresT / sqrt(R))                   (ScalarE, PSUM->SBUF)
       out_acc[n128, d512] += expT_tile.T @ v_tile    (PSUM accum over m)
       sum_acc[n128, 1]    += expT_tile.T @ ones
     out = out_acc * (1 / sum_acc)   (softmax normalization folded at the end)

Scheduling: one flat software-pipelined schedule across all chunks — the
scores/exp of couple k+1 are emitted before the AV matmuls of couple k, so
the ScalarE exp latency and the scores weight loads hide under AV work, and
chunk boundaries never drain the pipeline. The vpT quarters and uT chunks
are emitted right before the first couple that consumes them, overlapping
the kT/qT DMA with chunk-0/1-3 AV work. Dummy warm-up matmuls during the
DMA lead-in hold the PE's HAM clock gate at full rate before real work
arrives.
"""

import numpy as np

B, N, D, R = 4, 4096, 1024, 32
NLOC = N // 2            # rows per core
RSCALE = float(1.0 / np.sqrt(np.float32(R)))

N_CHUNK = 256            # rows of scores computed per PSUM round
D_HALF = 512             # PSUM bank width in fp32

LAST_RESULT = None       # test.py reads exec_time_ns etc. from here


def _build():
    from concourse import bacc, mybir
    from concourse.tile import TileContext

    f32 = mybir.dt.float32
    f16 = mybir.dt.float16
    EXP = mybir.ActivationFunctionType.Exp

    nc = bacc.Bacc("TRN2", target_bir_lowering=False)

    qT = nc.dram_tensor("qT", [D, NLOC], f16, kind="ExternalInput")
    kT = nc.dram_tensor("kT", [D, N], f16, kind="ExternalInput")
    v = nc.dram_tensor("v", [N, D], f16, kind="ExternalInput")
    wu = nc.dram_tensor("wu", [D, R], f16, kind="ExternalInput")
    wv = nc.dram_tensor("wv", [D, R], f16, kind="ExternalInput")
    o = nc.dram_tensor("o", [NLOC, D], f32, kind="ExternalOutput")

    DT = D // 128         # 8 d-tiles
    UC = NLOC // 512      # 4 uT column-chunks
    MQ = N // 1024        # 4 kT quarters
    NCH = NLOC // N_CHUNK  # 8 main-loop chunks
    MT = N // 128         # 32 m tiles
    VG = 8                # v row-groups of 512
    VPG = N // VG // 128  # 4 m-tiles per v group

    with TileContext(nc) as tc:
        with tc.tile_pool(name="singles", bufs=1) as singles, \
             tc.tile_pool(name="ktp", bufs=4) as ktp, \
             tc.tile_pool(name="qtp", bufs=4) as qtp, \
             tc.tile_pool(name="vpool", bufs=VG - 1) as vpool, \
             tc.tile_pool(name="expp", bufs=6) as expp, \
             tc.tile_pool(name="outp", bufs=3) as outp, \
             tc.tile_pool(name="rpool", bufs=4) as rpool, \
             tc.tile_pool(name="pacc", bufs=5, space="PSUM") as pacc, \
             tc.tile_pool(name="pscore", bufs=2, space="PSUM") as pscore, \
             tc.tile_pool(name="psums", bufs=1, space="PSUM") as psums:

            # ---- constants / projection weights ----
            wu_sb = singles.tile([128, DT, R], f16, tag="wu")
            nc.sync.dma_start(out=wu_sb, in_=wu.rearrange("(t p) r -> p t r", p=128))
            wv_sb = singles.tile([128, DT, R], f16, tag="wv")
            nc.sync.dma_start(out=wv_sb, in_=wv.rearrange("(t p) r -> p t r", p=128))
            ones = singles.tile([128, 2], f16, tag="ones")
            nc.vector.memset(ones, 1.0)

            # uT is replicated on partition groups 0-31 / 32-63 and vpT is
            # distributed even/odd-m-tile across them, so the scores matmuls
            # for an m-tile PAIR run concurrently as two row-tiled (K=32)
            # matmuls on different 32-row strips of the PE array.
            uT = singles.tile([2 * R, NLOC], f16, tag="uT")
            vpT = singles.tile([2 * R, MT // 2, 128], f16, tag="vpT")

            # ---- DMA emission, in order of need (one trigger per load:
            # DMA trigger instructions serialize at ~650ns on the Sync queue)
            def load_qt_chunk(c):
                # two d-halves, so the u projection can start on the first
                # half while the second streams in
                halves = []
                for h in range(2):
                    tile = qtp.tile([128, DT // 2, 512], f16, tag="qt",
                                    name=f"qt{c}_{h}")
                    d0 = h * (D // 2)
                    nc.sync.dma_start(
                        out=tile,
                        in_=qT[d0:d0 + D // 2,
                               c * 512:(c + 1) * 512].rearrange(
                            "(t p) c -> p t c", p=128))
                    halves.append(tile)
                return halves

            def load_kt_quarter(q):
                # kT arrives host-permuted: within each 1024-col quarter, the
                # even m-tiles' 512 columns come first, then the odd ones, so
                # the vp projection's even/odd moving operands are plain
                # slices. Loaded as two d-halves so the first projection
                # matmuls start after 1MB instead of 2MB.
                halves = []
                for h in range(2):
                    tile = ktp.tile([128, DT // 2, 1024], f16, tag="kt",
                                    name=f"kt{q}_{h}")
                    d0 = h * (D // 2)
                    nc.sync.dma_start(
                        out=tile,
                        in_=kT[d0:d0 + D // 2,
                               q * 1024:(q + 1) * 1024].rearrange(
                            "(t p) c -> p t c", p=128))
                    halves.append(tile)
                return halves

            v_sb = [None] * VG

            def load_v(g):
                vt = vpool.tile([128, VPG, D], f16, tag="v", name=f"v{g}")
                nc.sync.dma_start(
                    out=vt, in_=v[g * 512:(g + 1) * 512, :].rearrange(
                        "(t p) d -> p t d", p=128))
                v_sb[g] = vt

            # group 0 is split in half so the first AV matmul only waits on
            # 0.5MB of v
            def load_v0():
                va = vpool.tile([128, 2, D], f16, tag="v0a", name="v0a", bufs=1)
                nc.sync.dma_start(
                    out=va, in_=v[0:256, :].rearrange("(t p) d -> p t d", p=128))
                vb = vpool.tile([128, 2, D], f16, tag="v0b", name="v0b", bufs=1)
                nc.sync.dma_start(
                    out=vb, in_=v[256:512, :].rearrange("(t p) d -> p t d", p=128))
                v_sb[0] = (va, vb)

            qt0 = load_qt_chunk(0)
            kt_q = [load_kt_quarter(0)]
            load_v0()
            load_v(1)
            kt_q.append(load_kt_quarter(1))
            load_v(2)
            load_v(3)
            kt_q.append(load_kt_quarter(2))
            load_v(4)
            load_v(5)
            kt_q.append(load_kt_quarter(3))
            load_v(6)
            load_v(7)
            qt_rest = [load_qt_chunk(c) for c in (1, 2, 3)]

            def v_tile(mt):
                # returns the (low, high) D-half slices of the v row-tile mt
                g, tg = mt // VPG, mt % VPG
                if g == 0:
                    vt, tg = v_sb[0][tg // 2], tg % 2
                else:
                    vt = v_sb[g]
                return vt[:, tg, 0:D_HALF], vt[:, tg, D_HALF:D]

            # ---- PE warm-up: the HAM clock gate defaults to half rate and
            # only releases after ~3.4us of sustained PE activity. Dummy
            # matmuls (no DMA dependency) fill the otherwise-idle DMA lead-in
            # so the real work starts at full clock.
            junk = singles.tile([128, 512], f16, tag="junk")
            nc.vector.memset(junk, 0.0)
            wps = pscore.tile([128, 512], f32, tag="scores", name="warm")
            for _ in range(12):
                nc.tensor.matmul(wps, lhsT=junk[:, 0:128], rhs=junk,
                                 start=True, stop=True, skip_group_check=True)

            # ---- projection emitters ----
            # plain base-0 matmuls; the strip-1 destinations are written by
            # partition-shifted DVE copies (PSUM parts 0-31 -> SBUF 32-63)
            def u_chunk(c, qtile):
                qa, qb = qtile
                pu = pscore.tile([R, 512], f32, tag="scores", name=f"pu{c}")
                for t in range(DT):
                    qh = qa if t < DT // 2 else qb
                    nc.tensor.matmul(pu, lhsT=wu_sb[:, t, :],
                                     rhs=qh[:, t % (DT // 2), :],
                                     start=(t == 0), stop=(t == DT - 1))
                nc.vector.tensor_copy(out=uT[0:R, c * 512:(c + 1) * 512],
                                      in_=pu)
                nc.vector.tensor_copy(out=uT[R:2 * R, c * 512:(c + 1) * 512],
                                      in_=pu)

            def vp_quarter(qtr):
                kta, ktb = kt_q[qtr]
                pv0 = pscore.tile([R, 4, 128], f32, tag="scores",
                                  name=f"pv0_{qtr}")
                for t in range(DT):
                    kt = kta if t < DT // 2 else ktb
                    nc.tensor.matmul(pv0, lhsT=wv_sb[:, t, :],
                                     rhs=kt[:, t % (DT // 2), 0:512],
                                     start=(t == 0), stop=(t == DT - 1))
                nc.vector.tensor_copy(
                    out=vpT[0:R, qtr * 4:(qtr + 1) * 4, :], in_=pv0)
                pv1 = pscore.tile([R, 4, 128], f32, tag="scores",
                                  name=f"pv1_{qtr}")
                for t in range(DT):
                    kt = kta if t < DT // 2 else ktb
                    nc.tensor.matmul(pv1, lhsT=wv_sb[:, t, :],
                                     rhs=kt[:, t % (DT // 2), 512:1024],
                                     start=(t == 0), stop=(t == DT - 1))
                nc.vector.tensor_copy(
                    out=vpT[R:2 * R, qtr * 4:(qtr + 1) * 4, :], in_=pv1)

            u_chunk(0, qt0)
            vp_quarter(0)

            # ---- main loop: flash-style scores/softmax/AV ----
            # software-pipelined: scores/exp for m-tile mt+1 are issued before
            # the AV matmuls of m-tile mt, so ScalarE exp latency hides under
            # the previous tile's AV work on the PE. The pipeline never crosses
            # a segment boundary (chunk 0's segments are separated by vp
            # emissions whose matmuls must precede the dependent scores).
            def scores_quad(ch, cp):
                # a COUPLE of m-tile pairs (pairs 2cp, 2cp+1): four row-tiled
                # K=32 matmuls. Different row strips run concurrently on
                # different 32-row strips of the PE array; each strip gets its
                # OWN psum bank (mixed-strip writes into one bank crash the
                # exec unit), with the two same-strip pair outputs sharing
                # that bank at column offsets 0/256.
                ps = [pscore.tile([128, 2 * N_CHUNK], f32, tag="scores",
                                  name=f"ps{g}_{ch}_{cp}") for g in range(2)]
                for hp in range(2):
                    p = 2 * cp + hp
                    for g in range(2):
                        nc.tensor.matmul(
                            ps[g][:, hp * N_CHUNK:(hp + 1) * N_CHUNK],
                            lhsT=vpT[g * R:(g + 1) * R, p, :],
                            rhs=uT[g * R:(g + 1) * R,
                                   ch * N_CHUNK:(ch + 1) * N_CHUNK],
                            start=True, stop=True, skip_group_check=True)
                exs = []
                for hp in range(2):
                    ex = expp.tile([128, 2 * N_CHUNK], f16, tag="ex",
                                   name=f"ex{ch}_{2 * cp + hp}")
                    for g in range(2):
                        nc.scalar.activation(
                            out=ex[:, g * N_CHUNK:(g + 1) * N_CHUNK],
                            in_=ps[g][:, hp * N_CHUNK:(hp + 1) * N_CHUNK],
                            func=EXP, scale=RSCALE)
                    exs.append(ex)
                return exs

            # ---- flattened global schedule ----
            # scores are emitted in bursts of 3 (the pscore pool depth), two
            # groups ahead of the AV consumption: a dense LDW/MM burst
            # pipelines the vpT weight loads, instead of paying an unhidden
            # weight-load serialization against the AV stream on every m-tile
            # (the 32-row scores weights conflict with in-flight full-array
            # matmuls, so their loads cannot be hoisted by the PE reorder
            # window). The schedule runs across chunk boundaries so the next
            # chunk's scores pipeline is primed before the previous chunk
            # drains. Projection work (vpT quarters, uT chunks) is emitted
            # right before the first burst that consumes it.
            # u chunks go mid-chunk (couple 4), away from the busy chunk
            # boundary (norm DVE + next chunk's first couple)
            inserts = {
                (0, 2): lambda: vp_quarter(1),
                (0, 4): lambda: vp_quarter(2),
                (0, 6): lambda: vp_quarter(3),
                (1, 4): lambda: u_chunk(1, qt_rest[0]),
                (2, 4): lambda: u_chunk(2, qt_rest[1]),
                (3, 4): lambda: u_chunk(3, qt_rest[2]),
            }
            seq = [(ch, cp) for ch in range(NCH) for cp in range(MT // 4)]
            accs_by_ch = {}
            sums_by_ch = {}

            def get_acc(ch):
                # both sums accumulators share one bank: start=True clears
                # has_written bank-wide, so ONLY sums[0]'s first matmul
                # carries start=True; the cleared has_written makes sums[1]'s
                # first start=False matmul overwrite rather than accumulate
                if ch not in accs_by_ch:
                    accs_by_ch[ch] = [
                        pacc.tile([128, D_HALF], f32, tag="acc",
                                  name=f"acc{ch}_{i}") for i in range(4)]
                    st = psums.tile([128, 4], f32, tag="sums", name=f"sum{ch}")
                    sums_by_ch[ch] = [st[:, 0:2], st[:, 2:4]]
                return accs_by_ch[ch], sums_by_ch[ch]

            def norm_out(ch):
                accs, sums = accs_by_ch[ch], sums_by_ch[ch]
                for j in range(2):
                    rc = rpool.tile([128, 1], f32, tag="rc", name=f"rc{ch}_{j}")
                    nc.vector.reciprocal(rc, sums[j][:, 0:1])
                    ob = outp.tile([128, D], f32, tag="ob", name=f"ob{ch}_{j}")
                    nc.vector.tensor_scalar_mul(ob[:, 0:D_HALF], accs[2 * j], rc)
                    nc.vector.tensor_scalar_mul(ob[:, D_HALF:D],
                                                accs[2 * j + 1], rc)
                    row = ch * N_CHUNK + j * 128
                    nc.sync.dma_start(out=o[row:row + 128, :], in_=ob)

            ex_q = []

            def emit_couple(items):
                for ch, cp in items:
                    if (ch, cp) in inserts:
                        inserts.pop((ch, cp))()
                    ex_q.extend(scores_quad(ch, cp))

            emit_couple(seq[0:1])
            for i in range(len(seq)):
                emit_couple(seq[i + 1:i + 2])
                ch, cp = seq[i]
                accs, sums = get_acc(ch)
                for hp in range(2):
                    ex = ex_q.pop(0)
                    for sub in range(2):
                        mt = 4 * cp + 2 * hp + sub
                        vlo, vhi = v_tile(mt)
                        first, last = (mt == 0), (mt == MT - 1)
                        for j in range(2):
                            lhs = ex[:, sub * N_CHUNK + j * 128:
                                     sub * N_CHUNK + (j + 1) * 128]
                            nc.tensor.matmul(accs[2 * j], lhsT=lhs, rhs=vlo,
                                             start=first, stop=last)
                            nc.tensor.matmul(accs[2 * j + 1], lhsT=lhs,
                                             rhs=vhi, start=first, stop=last)
                            nc.tensor.matmul(sums[j], lhsT=lhs, rhs=ones,
                                             start=(first and j == 0),
                                             stop=last, skip_group_check=True)
                if cp == MT // 4 - 1:
                    norm_out(ch)

    nc.finalize()
    return nc


def kernel(q, k, v, Wu, Wv):
    global LAST_RESULT
    from concourse import bass_utils

    nc = _build()

    # permute kT columns: within each 1024-col quarter, even m-tiles' columns
    # first, then odd ones (see load_kt_quarter)
    def perm_kt(kb):
        kTb = kb.T.reshape(D, N // 1024, 4, 2, 128).swapaxes(2, 3)
        return np.ascontiguousarray(kTb.reshape(D, N)).astype(np.float16)

    kTs = [perm_kt(k[b]) for b in range(B)]
    vs = [np.ascontiguousarray(v[b]).astype(np.float16) for b in range(B)]
    wu16 = np.ascontiguousarray(Wu).astype(np.float16)
    wv16 = np.ascontiguousarray(Wv).astype(np.float16)
    in_maps = []
    for core in range(8):
        b, h = core // 2, core % 2
        in_maps.append({
            "qT": np.ascontiguousarray(
                q[b].T[:, h * NLOC:(h + 1) * NLOC]).astype(np.float16),
            "kT": kTs[b],
            "v": vs[b],
            "wu": wu16,
            "wv": wv16,
        })

    res = bass_utils.run_bass_kernel_spmd(nc, in_maps, core_ids=list(range(8)))
    LAST_RESULT = res

    out = np.empty((B, N, D), dtype=np.float32)
    for core in range(8):
        b, h = core // 2, core % 2
        out[b, h * NLOC:(h + 1) * NLOC, :] = res.results[core]["o"]
    return out
